# revision 1
# baseline (speedup 1.0000x reference)
"""Trainium2 Bass kernel for nn_Cross_transformer (4-branch channel cross-attention
+ fused 1x1 conv + BN + residual + 3x3 conv + BN), data-parallel over 8 NeuronCores
(one batch sample per core).

Algebraic restructuring (the key to the memory roofline):
  q = Wa Xa, K_s = Ws Xs                       (C x N, C=48, N=36864)
  energy_s = q K_s^T = Wa (Xa Xs^T) Ws^T       -> Gram G_s = Xa Xs^T is only 48x48
  att_s = softmax(rowmax - energy_s)           (tiny)
  conv1x1(cat(out4)) = sum_s F_s att_s K_s = sum_s (F_s att_s Ws) Xs = sum_s P_s Xs
so the big tensors are touched exactly twice (Gram pass, P_s-apply pass) and the
[B,4C,H,W] concat intermediate never exists. Conv biases preceding train-mode BN
cancel exactly and are dropped. gamma_cam is folded into BN1's affine (assumes
gamma_cam >= 0; it is 0.5 here).

Numerics: bf16 activations / fp32 PSUM + BN stats (validated: ~0.4% rel err).
BN stats are batch-global -> two [48,4] AllReduces across the 8 cores.

Layout: "pair layout" [128, N/2]: rows 0-47 = channels for the first spatial
half, rows 64-111 = second half, so DVE/ACT run 96-of-128 lanes and matmuls use
tile_position row/col packing. Transposed (spatial-major) tiles for the Gram come
from the DMA xbar-transpose engine (bf16), fed by SWDGE cast-on-load DMAs.
"""

import numpy as np
import ml_dtypes
from dataclasses import dataclass

import concourse.bass as bass
import concourse.bacc as bacc
import concourse.mybir as mybir
import concourse.tile as tile
from concourse.tile import add_dep_helper

F32 = mybir.dt.float32
BF16 = mybir.dt.bfloat16
ALU = mybir.AluOpType
ACTF = mybir.ActivationFunctionType


@dataclass(frozen=True)
class Cfg:
    C: int = 48
    H: int = 192
    W: int = 192
    n_cores: int = 8
    rows_per_chunk: int = 2      # image rows per matmul chunk
    slab: int = 1152             # phase-1 transpose slab width (n elements)
    xbar_piece: int = 384        # elements per xbar-transpose call
    sp_pads: int = 0             # SP queue-rotation pad copies per stripe
    eps: float = 1e-5

    @property
    def N(self):
        return self.H * self.W

    @property
    def CH(self):
        return self.rows_per_chunk * self.W

    @property
    def NCH(self):
        return self.N // self.CH

    @property
    def NHALF(self):
        return self.N // 2

    @property
    def NGRP(self):
        return self.NCH // 2

    @property
    def Wp(self):
        return self.W + 2

    @property
    def Hp(self):
        return self.H + 2

    @property
    def NPAD(self):              # padded image + two slack rows for view math
        return (self.Hp + 2) * self.Wp


def _fix_xpose_waits(nc):
    """DMA_DIRECT2D_XPOSE supports exactly one embedded sync wait (walrus
    asserts). Hoist excess waits from each transpose onto the nearest
    preceding SP-queue DMACopy (which lowers waits without that limit).
    Only waits whose awaited completion belongs to an instruction scheduled
    BEFORE that carrier are hoisted (tick snapshots prove no cycle)."""
    for bb in nc.m.functions[0].blocks:
        ticks = {}
        carrier = None
        snap = None
        for inst in bb.instructions:
            si = inst.sync_info
            nm = type(inst).__name__
            if nm == "InstDMACopy" and str(inst.engine).endswith("SP"):
                carrier, snap = inst, dict(ticks)
            elif (nm == "InstDmaTransposeAnt" and si and si.on_wait
                  and len(si.on_wait) > 1):
                keep, move = [], []
                for w in si.on_wait:
                    if carrier is not None and w.wait_value <= snap.get(w.id, 0):
                        move.append(w)
                    else:
                        keep.append(w)
                if len(keep) > 1:
                    raise RuntimeError(
                        f"{inst.name}: {len(keep)} unhoistable xpose waits: "
                        f"{[(w.id, w.wait_value) for w in keep]}")
                if move:
                    csi = carrier.sync_info
                    merged = {w.id: w for w in (csi.on_wait or [])}
                    for w in move:
                        if w.id not in merged or merged[w.id].wait_value < w.wait_value:
                            merged[w.id] = w
                    csi.on_wait = list(merged.values())
                    si.on_wait = keep
            if si and si.on_update:
                for u in si.on_update:
                    ticks[u.id] = ticks.get(u.id, 0) + (u.update_value or 1)


def build_kernel(nc, cfg: Cfg):
    C, N = cfg.C, cfg.N
    S, CS = 4, 4 * cfg.C
    assert cfg.NHALF % cfg.slab == 0 and cfg.slab % 128 == 0
    assert (cfg.NCH // 2) % 2 == 0

    xs_hbm = [nc.dram_tensor(f"x{s}", [C, N], F32, kind="ExternalInput").ap()
              for s in range(S)]
    consts_hbm = {
        "waT": nc.dram_tensor("waT", [C, C], F32, kind="ExternalInput").ap(),
        "wcatT": nc.dram_tensor("wcatT", [C, CS], F32, kind="ExternalInput").ap(),
        "wcat": nc.dram_tensor("wcat", [C, CS], F32, kind="ExternalInput").ap(),
        "fcatT": nc.dram_tensor("fcatT", [C, CS], F32, kind="ExternalInput").ap(),
        "eye": nc.dram_tensor("eye", [C, C], F32, kind="ExternalInput").ap(),
        "tapsT": nc.dram_tensor("tapsT", [128, 9 * C], BF16, kind="ExternalInput").ap(),
        "bn_gb": nc.dram_tensor("bn_gb", [C, 4], F32, kind="ExternalInput").ap(),
        "gvec": nc.dram_tensor("gvec", [128, 1], F32, kind="ExternalInput").ap(),
    }
    out_hbm = nc.dram_tensor("out", [C, N], F32, kind="ExternalOutput").ap()

    with tile.TileContext(nc, pool_alloc_mode="queue") as tc:
        _body(nc, tc, cfg, xs_hbm, consts_hbm, out_hbm)
    _fix_xpose_waits(nc)
    if isinstance(nc, bacc.Bacc):
        nc.compile()         # splits sync waits to >=1-per-instruction etc.
    return nc


def _body(nc, tc, cfg, xs_hbm, ch, out_hbm):
    C, N, CH = cfg.C, cfg.N, cfg.CH
    NGRP, NHALF = cfg.NGRP, cfg.NHALF
    NGH = NGRP // 2              # groups per spatial half
    S, CS = 4, 4 * cfg.C
    SLAB = cfg.slab
    NSLAB = N // SLAB
    KPS = SLAB // 128
    rpc, W, Wp, H = cfg.rows_per_chunk, cfg.W, cfg.Wp, cfg.H
    sync, gps, pe, dve, act = nc.sync, nc.gpsimd, nc.tensor, nc.vector, nc.scalar
    groups = [list(range(cfg.n_cores))]

    # ---------------- constants ----------------
    cpool = tc.alloc_tile_pool(name="consts", bufs=1)
    c_waT = cpool.tile([C, C], F32, name="c_waT")
    c_wcatT = cpool.tile([C, CS], F32, name="c_wcatT")
    c_wcat = cpool.tile([C, CS], F32, name="c_wcat")
    c_fcatT = cpool.tile([C, CS], F32, name="c_fcatT")
    c_eye = cpool.tile([C, C], F32, name="c_eye")
    c_taps = cpool.tile([128, 9 * C], BF16, name="c_taps")
    c_bn = cpool.tile([C, 4], F32, name="c_bn")
    c_gvec = cpool.tile([128, 1], F32, name="c_gvec")
    for t, key in ((c_waT, "waT"), (c_wcatT, "wcatT"), (c_wcat, "wcat"),
                   (c_fcatT, "fcatT"), (c_eye, "eye"), (c_taps, "tapsT"),
                   (c_bn, "bn_gb"), (c_gvec, "gvec")):
        gps.dma_start(t[:], ch[key])

    # DRAM bounce buffers for the two stat AllReduces
    dpool = tc.alloc_tile_pool(name="ccdram", bufs=1, space="DRAM")
    cc_in = [dpool.tile([C, 4], F32, name=f"cc_in{i}") for i in range(2)]
    cc_space = "Shared" if cfg.n_cores > 4 else "Local"
    cc_out = [dpool.tile([C, 4], F32, name=f"cc_out{i}", addr_space=cc_space)
              for i in range(2)]
    y_dram = dpool.tile([128, NHALF], BF16, name="y_dram")

    # scratch pool for small/stat tiles (lives for the whole kernel)
    tm = tc.alloc_tile_pool(name="tm", bufs=1)

    # ---------------- residents (pair layout) ----------------
    # one tile per n-stripe holding all 4 branches x both spatial halves, so a
    # transpose's (tensor-granular) source deps hit exactly this stripe's loads
    NIT = NHALF // SLAB
    pool_x = tc.alloc_tile_pool(name="xres", bufs=1)
    xsl = [pool_x.tile([128, S * SLAB], BF16, name=f"xsl{i}") for i in range(NIT)]

    def xs_at(s, half, off, ln):
        it, o = off // SLAB, off % SLAB
        return xsl[it][64 * half:64 * half + C, s * SLAB + o:s * SLAB + o + ln]

    # ============ PHASE 1: cast-load + xbar transpose + Gram ============
    pool_gp = tc.alloc_tile_pool(name="gps", bufs=1, space="PSUM")
    g_psum = pool_gp.tile([C, CS], F32, name="g_psum")

    tpool = tc.alloc_tile_pool(name="tstage", bufs=2)
    piece = min(cfg.xbar_piece, SLAB)
    npc = SLAB // piece
    kpp = piece // 128
    SAMP = 2 * S * KPS           # one fence sample per transpose k-unit
    kk = 0
    prev_xbar = None             # order next stripe's copies after our xposes
    for it in range(NIT):
        for h in range(2):
            for s in range(S):
                ld = gps.dma_start(xsl[it][64 * h:64 * h + C,
                                           s * SLAB:(s + 1) * SLAB],
                                   xs_hbm[s][:, h * NHALF + it * SLAB:
                                             h * NHALF + (it + 1) * SLAB])
                if prev_xbar is not None:
                    add_dep_helper(ld.ins, prev_xbar.ins, sync=False,
                                   reason="schedule after prev stripe xposes")
        F0 = 2 * KPS * CS
        tstg = tpool.tile([128, F0 + 16], BF16, name="tstg")
        tv = tstg[:, 0:F0].rearrange("p (h s k c) -> p h s k c", h=2, s=S, k=KPS)
        vb = tstg[:, 0:F0].rearrange("p (a b) -> p a b", b=C)
        spare = tstg[:, F0:F0 + 16]
        # DMA_DIRECT2D_XPOSE carries at most ONE sync wait (walrus limit), and
        # its dep tracking is tensor-granular. Two chained SP-DMA "fences"
        # (regular DMAs, no wait limit) read samples of every producer load's
        # range and write samples into every transpose-block of the recycled
        # staging slot: all producer/WAR/WAW ticks are observed by the SP
        # queue before the transposes issue, leaving each transpose a single
        # wait on the fence chain.
        s0 = xsl[it][0:C, :].rearrange("p (a b) -> p a b", a=SAMP)[:, :, 0]
        f0 = sync.dma_start(vb[0:C, :, C - 2], s0)
        s1 = xsl[it][64:64 + C, :].rearrange("p (a b) -> p a b", a=SAMP)[:, :, 0:2]
        f1 = sync.dma_start(vb[0:C, :, C - 2:C], s1)
        if prev_xbar is not None:
            add_dep_helper(f0.ins, prev_xbar.ins, sync=False,
                           reason="schedule after prev stripe xposes")
            add_dep_helper(f1.ins, prev_xbar.ins, sync=False,
                           reason="schedule after prev stripe xposes")
        # 7 dummy copies pad the SP HWDGE queue rotation so the first
        # transpose's own-queue predecessor IS f1: its single allowed wait
        # then covers both the fence WAW and the queue recycle.
        last_d = f1
        for dk in range(cfg.sp_pads):
            d = sync.dma_start(spare[0:1, 2 * dk:2 * dk + 2],
                               c_taps[0:1, 2 * dk:2 * dk + 2])
            add_dep_helper(d.ins, last_d.ins, sync=False,
                           reason="SP queue rotation pad")
            last_d = d
        first = True
        for h in range(2):
            for s in range(S):
                for pc in range(npc):
                    prev_xbar = sync.dma_start(
                        tv[:, h, s, pc * kpp:(pc + 1) * kpp, :],
                        xsl[it][64 * h:64 * h + C,
                                s * SLAB + pc * piece:s * SLAB + (pc + 1) * piece],
                        transpose=True)
                    if first:
                        add_dep_helper(prev_xbar.ins, last_d.ins, sync=False,
                                       reason="SP queue rotation pad")
                        first = False
        for h in range(2):
            for k in range(KPS):
                pe.matmul(g_psum[:], lhsT=tv[:, h, 0, k, :], rhs=tv[:, h, :, k, :],
                          start=(kk == 0), stop=(kk == 2 * NIT * KPS - 1))
                kk += 1
    tpool.release()

    # ============ tiny math: G -> P_s^T ============
    tmp = tc.alloc_tile_pool(name="tinyp", bufs=1, space="PSUM")
    g_sb = tm.tile([C, CS], F32, name="g_sb")
    dve.tensor_copy(g_sb[:], g_psum[:])

    v_ps = tmp.tile([C, CS], F32, name="v_ps")
    pe.matmul(v_ps[:], lhsT=c_waT[:], rhs=g_sb[:], start=True, stop=True)
    v_sb = tm.tile([C, CS], F32, name="v_sb")
    dve.tensor_copy(v_sb[:], v_ps[:])

    vt_sb = tm.tile([C, CS], F32, name="vt_sb")
    for s in range(S):
        vt_ps = tmp.tile([C, C], F32, name="vt_ps", tag="t48")
        pe.transpose(vt_ps[:], v_sb[:, s * C:(s + 1) * C], c_eye[:])
        dve.tensor_copy(vt_sb[:, s * C:(s + 1) * C], vt_ps[:])

    e_ps = tmp.tile([C, CS], F32, name="e_ps")
    for s in range(S):
        pe.matmul(e_ps[:, s * C:(s + 1) * C],
                  lhsT=vt_sb[:, s * C:(s + 1) * C],
                  rhs=c_wcatT[:, s * C:(s + 1) * C], start=True, stop=True)
    e_sb = tm.tile([C, CS], F32, name="e_sb")
    dve.tensor_copy(e_sb[:], e_ps[:])

    # softmax(rowmax - E) over d == softmax(-E): p = exp(rmin - E), normalize
    rmin = tm.tile([C, S], F32, name="rmin")
    dve.tensor_reduce(rmin[:], e_sb[:].rearrange("c (s d) -> c s d", s=S),
                      mybir.AxisListType.X, ALU.min)
    p_sb = tm.tile([C, CS], F32, name="p_sb")
    for s in range(S):
        act.activation(p_sb[:, s * C:(s + 1) * C], e_sb[:, s * C:(s + 1) * C],
                       ACTF.Exp, bias=rmin[:, s:s + 1], scale=-1.0)
    pssum = tm.tile([C, S], F32, name="pssum")
    dve.tensor_reduce(pssum[:], p_sb[:].rearrange("c (s d) -> c s d", s=S),
                      mybir.AxisListType.X, ALU.add)
    prec = tm.tile([C, S], F32, name="prec")
    dve.reciprocal(prec[:], pssum[:])
    att_sb = tm.tile([C, CS], F32, name="att_sb")
    for s in range(S):
        dve.tensor_scalar_mul(att_sb[:, s * C:(s + 1) * C],
                              p_sb[:, s * C:(s + 1) * C], prec[:, s:s + 1])

    # P_s^T = Ws^T att_s^T F_s^T
    pT_hi = tm.tile([128, CS], BF16, name="pT_hi")
    for s in range(S):
        q1_ps = tmp.tile([C, C], F32, name="q1_ps", tag="t48")
        pe.matmul(q1_ps[:], lhsT=att_sb[:, s * C:(s + 1) * C],
                  rhs=c_fcatT[:, s * C:(s + 1) * C], start=True, stop=True)
        q1_sb = tm.tile([C, C], F32, name="q1_sb", bufs=2)
        dve.tensor_copy(q1_sb[:], q1_ps[:])
        p_ps = tmp.tile([C, C], F32, name="p_ps", tag="t48")
        pe.matmul(p_ps[:], lhsT=c_wcat[:, s * C:(s + 1) * C], rhs=q1_sb[:],
                  start=True, stop=True)
        dve.tensor_copy(pT_hi[0:C, s * C:(s + 1) * C], p_ps[:])
    tmp.release()
    pool_gp.release()
    sync.dma_start(pT_hi[64:64 + C, :], pT_hi[0:C, :])

    # ============ PHASE 2: y = sum_s P_s X_s (+ bn stats) ============
    stats1 = tm.tile([128, NGRP * 6], F32, name="stats1")
    gps.memset(stats1[:], 0.0)
    ytpool = tc.alloc_tile_pool(name="ytp", bufs=3, side="right")
    yps = tc.alloc_tile_pool(name="yps", bufs=3, space="PSUM")
    for g in range(NGRP):
        half = 0 if g < NGH else 1
        row = 64 * half
        j = (g - half * NGH) * 2
        yp = {0: yps.tile([128, CH], F32, name="yp0", tag="yp0"),
              64: yps.tile([128, CH], F32, name="yp1", tag="yp1")}
        for s in range(S):
            for col, jj in ((0, j), (64, j + 1)):
                pe.matmul(yp[col][col:col + C, :],
                          lhsT=pT_hi[row:row + C, s * C:(s + 1) * C],
                          rhs=xs_at(s, half, jj * CH, CH),
                          start=(s == 0), stop=(s == S - 1),
                          tile_position=(row, col))
        yt = ytpool.tile([128, CH], BF16, name="yt")
        dve.memset(yt[32:64, :], 0.0)
        dve.memset(yt[96:128, :], 0.0)
        eng = act if g % 2 == 0 else dve
        eng.activation(yt[0:C, :], yp[0][0:C, :], ACTF.Copy) if eng is act else             eng.tensor_copy(yt[0:C, :], yp[0][0:C, :])
        eng.activation(yt[64:64 + C, :], yp[64][64:64 + C, :], ACTF.Copy)             if eng is act else eng.tensor_copy(yt[64:64 + C, :], yp[64][64:64 + C, :])
        dve.bn_stats(stats1[0:112, g * 6:(g + 1) * 6], yt[0:112, :])
        sync.dma_start(y_dram[:, g * CH:(g + 1) * CH], yt[:])
    yps.release()
    ytpool.release()

    # ============ AllReduce 1 -> BN1 affine (gamma_cam folded in) ============
    s1, b1 = _bn_allreduce(nc, tc, tm, cfg, stats1, cc_in[0][:], cc_out[0][:],
                           groups, c_bn[:, 0:1], c_bn[:, 1:2], gamma=c_gvec)

    # X_a regrouped to match the y/out_pre group layout (for the residual)
    pool_xg = tc.alloc_tile_pool(name="xgrp", bufs=1, side="right")
    xa_g = pool_xg.tile([128, NHALF], BF16, name="xa_g")
    gps.memset(xa_g[32:64, :], 0.0)
    gps.memset(xa_g[96:128, :], 0.0)
    NCHH = NHALF // CH
    for half in range(2):
        for jj in range(NCHH):
            g = half * NGH + jj // 2
            prow = 64 * (jj % 2)
            cpy = sync.dma_start(xa_g[prow:prow + C, g * CH:(g + 1) * CH],
                                 xs_at(0, half, jj * CH, CH))
            # keep HWDGE copies from hoisting between phase-1 fences/xposes
            add_dep_helper(cpy.ins, prev_xbar.ins, sync=False,
                           reason="schedule after phase-1 xposes")
    pool_x.release()
    pool_op = tc.alloc_tile_pool(name="outpre", bufs=1)
    op_sb = pool_op.tile([128, NHALF], BF16, name="op_sb")

    # ============ PHASE 3: out_pre = relu(s1*y + b1) + X_a ============
    p3pool = tc.alloc_tile_pool(name="p3t", bufs=3, side="right")
    for g in range(NGRP):
        yt2 = p3pool.tile([128, CH], BF16, name="yt2")
        sync.dma_start(yt2[:], y_dram[:, g * CH:(g + 1) * CH])
        t = p3pool.tile([128, CH], F32, name="t3")
        act.activation(t[:], yt2[:], ACTF.Relu, bias=b1[:, 0:1], scale=s1[:, 0:1])
        dve.tensor_add(op_sb[:, g * CH:(g + 1) * CH], t[:],
                       xa_g[:, g * CH:(g + 1) * CH])
    p3pool.release()
    pool_xg.release()

    # ====== build two zero-padded images (partition rows 0-47 and 64-111) ====
    pool_pad = tc.alloc_tile_pool(name="pads", bufs=1)
    pad = pool_pad.tile([128, cfg.NPAD], BF16, name="pad")
    gps.memset(pad[0:C, :], 0.0)
    gps.memset(pad[64:64 + C, :], 0.0)
    pad3 = pad[:].rearrange("p (h w) -> p h w", w=Wp)    # h = Hp+2 rows (slack)
    opv = op_sb[:].rearrange("p (blk x) -> p blk x", x=CH)
    for drow in (0, 64):
        for half in range(2):
            for odd in range(2):
                for r in range(rpc):
                    srow = 64 * odd
                    g0 = half * NGH
                    src = opv[srow:srow + C, g0:g0 + NGH, r * W:(r + 1) * W]
                    # image row h = half*H/2 + (2g+odd)*rpc + r, +1 top pad
                    h0 = half * (H // 2) + odd * rpc + r + 1
                    dst = pad3[drow:drow + C, h0:h0 + 2 * rpc * NGH, 1:1 + W]
                    dst = dst.rearrange("c (g q) w -> c g q w", q=2 * rpc)
                    sync.dma_start(dst[:, :, 0:1, :], src)

    # ============ PHASE 4: conv3x3 (9 shifted matmuls) + bn stats ============
    pool_cv = tc.alloc_tile_pool(name="convsb", bufs=1)
    conv_sb = pool_cv.tile([128, NHALF], F32, name="conv_sb")
    gps.memset(conv_sb[32:64, :], 0.0)
    gps.memset(conv_sb[96:128, :], 0.0)
    stats2 = tm.tile([128, NGRP * 6], F32, name="stats2")
    gps.memset(stats2[:], 0.0)
    cps = tc.alloc_tile_pool(name="cps", bufs=3, space="PSUM")
    for g in range(NGRP):
        half = 0 if g < NGH else 1
        row = 64 * half
        j = (g - half * NGH) * 2
        h0 = half * (H // 2) + j * rpc
        cp = {0: cps.tile([128, CH], F32, name="cp0", tag="cp0"),
              64: cps.tile([128, CH], F32, name="cp1", tag="cp1")}
        for it in range(9):
            dy, dx = it // 3, it % 3
            for col, jj in ((0, 0), (64, 1)):
                rhs = pad3[row:row + C, h0 + jj * rpc + dy:h0 + jj * rpc + dy + rpc,
                           dx:dx + W]
                pe.matmul(cp[col][col:col + C, :],
                          lhsT=c_taps[row:row + C, it * C:(it + 1) * C],
                          rhs=rhs, start=(it == 0), stop=(it == 8),
                          tile_position=(row, col))
        for r0 in (0, 64):
            dve.bn_stats(stats2[r0:r0 + C, g * 6:(g + 1) * 6], cp[r0][r0:r0 + C, :])
            act.activation(conv_sb[r0:r0 + C, g * CH:(g + 1) * CH],
                           cp[r0][r0:r0 + C, :], ACTF.Copy)
    cps.release()

    # ============ AllReduce 2 -> BN2 affine ============
    s2, b2 = _bn_allreduce(nc, tc, tm, cfg, stats2, cc_in[1][:], cc_out[1][:],
                           groups, c_bn[:, 2:3], c_bn[:, 3:4], gamma=None)

    # ============ PHASE 5: out = relu(s2*conv + b2) -> HBM ============
    p5pool = tc.alloc_tile_pool(name="p5t", bufs=3)
    for g in range(NGRP):
        half = 0 if g < NGH else 1
        j = (g - half * NGH) * 2
        n0 = half * NHALF + j * CH
        o = p5pool.tile([128, CH], F32, name="o5")
        act.activation(o[:], conv_sb[:, g * CH:(g + 1) * CH], ACTF.Relu,
                       bias=b2[:, 0:1], scale=s2[:, 0:1])
        sync.dma_start(out_hbm[:, n0:n0 + CH], o[0:C, :])
        sync.dma_start(out_hbm[:, n0 + CH:n0 + 2 * CH], o[64:64 + C, :])
    p5pool.release()
    pool_cv.release()
    pool_pad.release()
    pool_op.release()
    tm.release()
    dpool.release()
    cpool.release()


def _bn_allreduce(nc, tc, tm, cfg, stats, cc_in, cc_out, groups, g_ap, b_ap,
                  gamma=None):
    """bn_stats blocks -> per-row (sum, sumsq) -> AllReduce over cores ->
    per-channel affine (scale, bias) replicated to rows 0-47 / 64-111."""
    C = cfg.C
    dve, act, gps, sync = nc.vector, nc.scalar, nc.gpsimd, nc.sync
    n_loc = cfg.NGRP * cfg.CH
    inv_ntot = 1.0 / float(cfg.n_cores * 2 * n_loc)
    uid = "1" if gamma is not None else "2"

    aggr = tm.tile([128, 2], F32, name=f"aggr{uid}")
    gps.memset(aggr[:], 0.0)
    dve.bn_aggr(aggr[0:C, :], stats[0:C, :])
    dve.bn_aggr(aggr[64:64 + C, :], stats[64:64 + C, :])
    ss = tm.tile([128, 2], F32, name=f"ss{uid}")
    dve.tensor_scalar_mul(ss[:, 0:1], aggr[:, 0:1], float(n_loc))
    msq = tm.tile([128, 1], F32, name=f"msq{uid}")
    dve.tensor_tensor(msq[:], aggr[:, 0:1], aggr[:, 0:1], ALU.mult)
    dve.tensor_tensor(ss[:, 1:2], aggr[:, 1:2], msq[:], ALU.add)
    dve.tensor_scalar_mul(ss[:, 1:2], ss[:, 1:2], float(n_loc))

    sync.dma_start(cc_in[:, 0:2], ss[0:C, :])
    sync.dma_start(cc_in[:, 2:4], ss[64:64 + C, :])
    gps.collective_compute("AllReduce", ALU.add, replica_groups=groups,
                           ins=[cc_in], outs=[cc_out])
    gsb = tm.tile([C, 4], F32, name=f"gsb{uid}")
    sync.dma_start(gsb[:], cc_out)

    mean = tm.tile([C, 1], F32, name=f"mean{uid}")
    dve.tensor_tensor(mean[:], gsb[:, 0:1], gsb[:, 2:3], ALU.add)
    dve.tensor_scalar_mul(mean[:], mean[:], inv_ntot)
    ex2 = tm.tile([C, 1], F32, name=f"ex2{uid}")
    dve.tensor_tensor(ex2[:], gsb[:, 1:2], gsb[:, 3:4], ALU.add)
    dve.tensor_scalar_mul(ex2[:], ex2[:], inv_ntot)
    msq2 = tm.tile([C, 1], F32, name=f"msq2{uid}")
    dve.tensor_tensor(msq2[:], mean[:], mean[:], ALU.mult)
    var = tm.tile([C, 1], F32, name=f"var{uid}")
    dve.tensor_tensor(var[:], ex2[:], msq2[:], ALU.subtract)
    dve.tensor_scalar_add(var[:], var[:], cfg.eps)
    sd = tm.tile([C, 1], F32, name=f"sd{uid}")
    act.activation(sd[:], var[:], ACTF.Sqrt)
    inv = tm.tile([C, 1], F32, name=f"inv{uid}")
    dve.reciprocal(inv[:], sd[:])

    sc = tm.tile([128, 1], F32, name=f"sc{uid}")
    bi = tm.tile([128, 1], F32, name=f"bi{uid}")
    gps.memset(sc[:], 0.0)
    gps.memset(bi[:], 0.0)
    dve.tensor_tensor(sc[0:C, :], inv[:], g_ap, ALU.mult)
    bt = tm.tile([C, 1], F32, name=f"bt{uid}")
    dve.tensor_tensor(bt[:], mean[:], sc[0:C, :], ALU.mult)
    dve.tensor_tensor(bi[0:C, :], b_ap, bt[:], ALU.subtract)
    if gamma is not None:        # fold gamma_cam (valid for gamma_cam >= 0)
        dve.tensor_tensor(sc[0:C, :], sc[0:C, :], gamma[0:C, :], ALU.mult)
        dve.tensor_tensor(bi[0:C, :], bi[0:C, :], gamma[0:C, :], ALU.mult)
    sync.dma_start(sc[64:64 + C, :], sc[0:C, :])
    sync.dma_start(bi[64:64 + C, :], bi[0:C, :])
    return sc, bi


# ======================= host side =======================

_CACHE = {}


def _prep_consts(cfg, inputs):
    C = cfg.C
    f32 = np.float32
    Wm = [np.asarray(inputs[k], f32) for k in ("Wa", "Wb", "Wc", "Wd")]
    fuse_w = np.asarray(inputs["fuse_w"], f32)[:, :, 0, 0]
    out_w = np.asarray(inputs["out_w"], f32)
    taps = np.zeros((128, 9 * C), np.float32)
    for t in range(9):
        dy, dx = t // 3, t % 3
        wT = out_w[:, :, dy, dx].T
        taps[0:C, t * C:(t + 1) * C] = wT
        taps[64:64 + C, t * C:(t + 1) * C] = wT
    bn_gb = np.stack([np.asarray(inputs["fuse_gamma"], f32),
                      np.asarray(inputs["fuse_beta"], f32),
                      np.asarray(inputs["out_gamma"], f32),
                      np.asarray(inputs["out_beta"], f32)], axis=1)
    gvec = np.full((128, 1), np.asarray(inputs["gamma_cam"], f32).reshape(-1)[0], f32)
    return {
        "waT": np.ascontiguousarray(Wm[0].T),
        "wcatT": np.ascontiguousarray(np.concatenate([w.T for w in Wm], 1)),
        "wcat": np.ascontiguousarray(np.concatenate(Wm, 1)),
        "fcatT": np.ascontiguousarray(
            np.concatenate([fuse_w[:, s * C:(s + 1) * C].T for s in range(4)], 1)),
        "eye": np.eye(C, dtype=f32),
        "tapsT": taps.astype(ml_dtypes.bfloat16),
        "bn_gb": np.ascontiguousarray(bn_gb),
        "gvec": gvec,
    }


def _get_built(cfg):
    if cfg not in _CACHE:
        nc = bacc.Bacc("TRN2", target_bir_lowering=False, debug=False,
                       enable_asserts=False, num_devices=cfg.n_cores)
        _CACHE[cfg] = build_kernel(nc, cfg)
    return _CACHE[cfg]


def kernel(**inputs):
    from concourse import bass_utils
    cfg = Cfg()
    nc = _get_built(cfg)
    consts = _prep_consts(cfg, inputs)
    B, C, H, W = cfg.n_cores, cfg.C, cfg.H, cfg.W
    xs = [np.asarray(inputs[k], np.float32).reshape(B, C, H * W)
          for k in ("input_feature", "fb", "fc", "fd")]
    in_maps = []
    for b in range(B):
        m = dict(consts)
        for s in range(4):
            m[f"x{s}"] = np.ascontiguousarray(xs[s][b])
        in_maps.append(m)
    res = bass_utils.run_bass_kernel_spmd(nc, in_maps, core_ids=list(range(B)))
    out = np.stack([res.results[b]["out"].reshape(C, H, W) for b in range(B)])
    return out.astype(np.float32)


if __name__ == "__main__":
    _get_built(Cfg())
    print("built OK")



# revision 13
# speedup vs baseline: 2.8766x; 2.8766x over previous
"""Trainium2 Bass kernel for nn_Cross_transformer (4-branch channel cross-attention
+ fused 1x1 conv + BN + residual + 3x3 conv + BN), data-parallel over 8 NeuronCores
(one batch sample per core).

Algebraic restructuring (the key to the memory roofline):
  q = Wa Xa, K_s = Ws Xs                       (C x N, C=48, N=36864)
  energy_s = q K_s^T = Wa (Xa Xs^T) Ws^T       -> Gram G_s = Xa Xs^T is only 48x48
  att_s = softmax(rowmax - energy_s)           (tiny)
  conv1x1(cat(out4)) = sum_s F_s att_s K_s = sum_s (F_s att_s Ws) Xs = sum_s P_s Xs
so the big tensors are touched exactly twice (Gram pass, P_s-apply pass) and the
[B,4C,H,W] concat intermediate never exists. Conv biases preceding train-mode BN
cancel exactly and are dropped. gamma_cam is folded into BN1's affine (gamma_cam
>= 0; it is 0.5 here).

v2 layout/schedule (vs v1):
- Host concatenates the 4 inputs into ONE bf16 HBM tensor xcat [4C, N]. The
  spatial-major tiles for the Gram come from big DMA xbar transposes DIRECTLY
  from HBM (one [192, SLAB] transpose per stripe-half, alternating the two
  HWDGE queues sync/scalar) - no staging, no fences, no per-op cast.
- Channel-major residents load concurrently via SWDGE (gpsimd), split into an
  xa pool (kept through phase 3 for the residual) and an xbcd pool (released
  after phase 2).
- Pair layout everywhere: spatial half 0 in partitions 0-47, half 1 in 64-111.
  Phases 2/4 run both halves CONCURRENTLY on the PE via tile_position (0,0) +
  (64,64) (different row groups stream in parallel, LDWEIGHTS pulls ahead).
- y stays in SBUF (no DRAM bounce). BN stats are taken straight from PSUM.
- Phase 3 writes relu(s1*y+b1)+xa directly into per-row-group half-image padded
  conv buffers (96+2 halo rows each); only 2 tiny cross-partition halo DMAs.
  Phase 4 (3x3 conv as 9 shifted matmuls) pipelines behind phase 3.
- Output staged in SBUF f32 and stored with a few large DMAs.

Numerics: bf16 activations / fp32 PSUM + BN stats. BN stats are batch-global ->
two [48,4] AllReduces across the 8 cores (DRAM bounce, gpsimd collectives).
"""

import numpy as np
import ml_dtypes
from dataclasses import dataclass

import concourse.bass as bass
import concourse.bacc as bacc
import concourse.mybir as mybir
import concourse.tile as tile

F32 = mybir.dt.float32
BF16 = mybir.dt.bfloat16
ALU = mybir.AluOpType
ACTF = mybir.ActivationFunctionType


@dataclass(frozen=True)
class Cfg:
    C: int = 48
    H: int = 192
    W: int = 192
    n_cores: int = 8
    slab: int = 2304             # phase-1 stripe width (elements per half)
    ch: int = 384                # chunk width = 2 image rows
    eps: float = 1e-5

    @property
    def N(self):
        return self.H * self.W

    @property
    def NHALF(self):
        return self.N // 2

    @property
    def NIT(self):
        return self.NHALF // self.slab

    @property
    def KPS(self):
        return self.slab // 128

    @property
    def NCH(self):               # chunk-pairs (each covers both halves)
        return self.NHALF // self.ch

    @property
    def Wp(self):
        return self.W + 2

    @property
    def PADR(self):              # slots per half-image pad: 96 rows + halos/zeros
        return self.H // 2 + 3

    @property
    def NPADG(self):
        return self.PADR * self.Wp


def build_kernel(nc, cfg: Cfg):
    C, N = cfg.C, cfg.N
    CS = 4 * C

    xcat_hbm = nc.dram_tensor("xcat", [CS, N], BF16, kind="ExternalInput").ap()
    consts_hbm = {
        "waT": nc.dram_tensor("waT", [C, C], F32, kind="ExternalInput").ap(),
        "wcatT": nc.dram_tensor("wcatT", [C, CS], F32, kind="ExternalInput").ap(),
        "wcat": nc.dram_tensor("wcat", [C, CS], F32, kind="ExternalInput").ap(),
        "fcatT": nc.dram_tensor("fcatT", [C, CS], F32, kind="ExternalInput").ap(),
        "eye": nc.dram_tensor("eye", [C, C], F32, kind="ExternalInput").ap(),
        "tapsT": nc.dram_tensor("tapsT", [128, 9 * C], BF16, kind="ExternalInput").ap(),
        "bn_gb": nc.dram_tensor("bn_gb", [C, 4], F32, kind="ExternalInput").ap(),
        "gvec": nc.dram_tensor("gvec", [128, 1], F32, kind="ExternalInput").ap(),
    }
    out_hbm = nc.dram_tensor("out", [C, N], F32, kind="ExternalOutput").ap()

    with tile.TileContext(nc, pool_alloc_mode="queue") as tc:
        _body(nc, tc, cfg, xcat_hbm, consts_hbm, out_hbm)
    if isinstance(nc, bacc.Bacc):
        nc.compile()
    return nc


def _body(nc, tc, cfg, xcat, ch, out_hbm):
    C, N, CH = cfg.C, cfg.N, cfg.ch
    NHALF, NIT, KPS, SLAB = cfg.NHALF, cfg.NIT, cfg.KPS, cfg.slab
    NCH = cfg.NCH
    S, CS = 4, 4 * cfg.C
    Wp, W, PADR = cfg.Wp, cfg.W, cfg.PADR
    CPS = SLAB // CH             # chunks per stripe
    sync, gps, pe, dve, act = nc.sync, nc.gpsimd, nc.tensor, nc.vector, nc.scalar
    groups = [list(range(cfg.n_cores))]
    hwq = [sync, act]            # the two HWDGE queues

    # ---------------- constants ----------------
    cpool = tc.alloc_tile_pool(name="consts", bufs=1)
    c_waT = cpool.tile([C, C], F32, name="c_waT")
    c_wcatT = cpool.tile([C, CS], F32, name="c_wcatT")
    c_wcat = cpool.tile([C, CS], F32, name="c_wcat")
    c_fcatT = cpool.tile([C, CS], F32, name="c_fcatT")
    c_eye = cpool.tile([C, C], F32, name="c_eye")
    c_taps = cpool.tile([128, 9 * C], BF16, name="c_taps")
    c_bn = cpool.tile([C, 4], F32, name="c_bn")
    c_gvec = cpool.tile([128, 1], F32, name="c_gvec")
    for t, key in ((c_waT, "waT"), (c_wcatT, "wcatT"), (c_wcat, "wcat"),
                   (c_fcatT, "fcatT"), (c_eye, "eye"), (c_taps, "tapsT"),
                   (c_bn, "bn_gb"), (c_gvec, "gvec")):
        gps.dma_start(t[:], ch[key])

    # DRAM bounce buffers for the two stat AllReduces
    dpool = tc.alloc_tile_pool(name="ccdram", bufs=1, space="DRAM")
    cc_in = [dpool.tile([C, 4], F32, name=f"cc_in{i}") for i in range(2)]
    cc_space = "Shared" if cfg.n_cores > 4 else "Local"
    cc_out = [dpool.tile([C, 4], F32, name=f"cc_out{i}", addr_space=cc_space)
              for i in range(2)]

    # scratch pool for small/stat tiles (lives for the whole kernel)
    tm = tc.alloc_tile_pool(name="tm", bufs=1)

    # ---------------- residents (pair layout) ----------------
    pool_xa = tc.alloc_tile_pool(name="xares", bufs=1)
    xa_t = [pool_xa.tile([128, SLAB], BF16, name=f"xa{i}") for i in range(NIT)]
    pool_xb = tc.alloc_tile_pool(name="xbres", bufs=1)
    xb_t = [pool_xb.tile([128, 3 * SLAB], BF16, name=f"xb{i}") for i in range(NIT)]

    def xs_at(s, half, g):
        it, o = g // CPS, (g % CPS) * CH
        r = 64 * half
        if s == 0:
            return xa_t[it][r:r + C, o:o + CH]
        return xb_t[it][r:r + C, (s - 1) * SLAB + o:(s - 1) * SLAB + o + CH]

    # ============ PHASE 1: HBM xbar-transposes + Gram;  SWDGE loads ============
    pool_gp = tc.alloc_tile_pool(name="gps", bufs=1, space="PSUM")
    g_psum = pool_gp.tile([C, CS], F32, name="g_psum")

    tvp = tc.alloc_tile_pool(name="tstage", bufs=2)
    NSTR = 2 * NIT
    kk = 0
    for st in range(NSTR):
        it, h = st // 2, st % 2
        off = h * NHALF + it * SLAB
        # spatial-major tiles straight from HBM: tv[p, k, c] = xcat[c, off+128k+p]
        tv = tvp.tile([128, KPS * CS], BF16, name="tv")
        tv3 = tv[:].rearrange("p (k c) -> p k c", c=CS)
        hwq[st % 2].dma_start(tv3, xcat[:, off:off + SLAB], transpose=True)
        # channel-major residents (SWDGE, overlapped; consumed in phase 2+)
        gps.dma_start(xa_t[it][64 * h:64 * h + C, :], xcat[0:C, off:off + SLAB])
        gps.dma_start(
            xb_t[it][64 * h:64 * h + C, :].rearrange("p (s n) -> p s n", s=3),
            xcat[C:CS, off:off + SLAB].rearrange("(s c) n -> c s n", s=3))
        for k in range(KPS):
            pe.matmul(g_psum[:], lhsT=tv3[:, k, 0:C], rhs=tv3[:, k, :],
                      start=(kk == 0), stop=(kk == NSTR * KPS - 1))
            kk += 1
    tvp.release()

    # ============ tiny math: G -> P_s^T ============
    tmp = tc.alloc_tile_pool(name="tinyp", bufs=1, space="PSUM")
    g_sb = tm.tile([C, CS], F32, name="g_sb")
    dve.tensor_copy(g_sb[:], g_psum[:])

    v_ps = tmp.tile([C, CS], F32, name="v_ps")
    pe.matmul(v_ps[:], lhsT=c_waT[:], rhs=g_sb[:], start=True, stop=True)
    v_sb = tm.tile([C, CS], F32, name="v_sb")
    dve.tensor_copy(v_sb[:], v_ps[:])

    vt_sb = tm.tile([C, CS], F32, name="vt_sb")
    for s in range(S):
        vt_ps = tmp.tile([C, C], F32, name="vt_ps", tag="t48")
        pe.transpose(vt_ps[:], v_sb[:, s * C:(s + 1) * C], c_eye[:])
        dve.tensor_copy(vt_sb[:, s * C:(s + 1) * C], vt_ps[:])

    e_ps = tmp.tile([C, CS], F32, name="e_ps")
    for s in range(S):
        pe.matmul(e_ps[:, s * C:(s + 1) * C],
                  lhsT=vt_sb[:, s * C:(s + 1) * C],
                  rhs=c_wcatT[:, s * C:(s + 1) * C], start=True, stop=True)
    e_sb = tm.tile([C, CS], F32, name="e_sb")
    dve.tensor_copy(e_sb[:], e_ps[:])

    # softmax(rowmax - E) over d == softmax(-E): p = exp(rmin - E), normalize
    rmin = tm.tile([C, S], F32, name="rmin")
    dve.tensor_reduce(rmin[:], e_sb[:].rearrange("c (s d) -> c s d", s=S),
                      mybir.AxisListType.X, ALU.min)
    p_sb = tm.tile([C, CS], F32, name="p_sb")
    for s in range(S):
        act.activation(p_sb[:, s * C:(s + 1) * C], e_sb[:, s * C:(s + 1) * C],
                       ACTF.Exp, bias=rmin[:, s:s + 1], scale=-1.0)
    pssum = tm.tile([C, S], F32, name="pssum")
    dve.tensor_reduce(pssum[:], p_sb[:].rearrange("c (s d) -> c s d", s=S),
                      mybir.AxisListType.X, ALU.add)
    prec = tm.tile([C, S], F32, name="prec")
    dve.reciprocal(prec[:], pssum[:])
    att_sb = tm.tile([C, CS], F32, name="att_sb")
    for s in range(S):
        dve.tensor_scalar_mul(att_sb[:, s * C:(s + 1) * C],
                              p_sb[:, s * C:(s + 1) * C], prec[:, s:s + 1])

    # P_s^T = Ws^T att_s^T F_s^T
    pT_hi = tm.tile([128, CS], BF16, name="pT_hi")
    for s in range(S):
        q1_ps = tmp.tile([C, C], F32, name="q1_ps", tag="t48")
        pe.matmul(q1_ps[:], lhsT=att_sb[:, s * C:(s + 1) * C],
                  rhs=c_fcatT[:, s * C:(s + 1) * C], start=True, stop=True)
        q1_sb = tm.tile([C, C], F32, name="q1_sb", bufs=2)
        dve.tensor_copy(q1_sb[:], q1_ps[:])
        p_ps = tmp.tile([C, C], F32, name="p_ps", tag="t48")
        pe.matmul(p_ps[:], lhsT=c_wcat[:, s * C:(s + 1) * C], rhs=q1_sb[:],
                  start=True, stop=True)
        dve.tensor_copy(pT_hi[0:C, s * C:(s + 1) * C], p_ps[:])
    tmp.release()
    pool_gp.release()
    sync.dma_start(pT_hi[64:64 + C, :], pT_hi[0:C, :])

    # ============ PHASE 2: y = sum_s P_s X_s (+ bn stats from PSUM) ============
    # y_sb holds y (bf16) through phase 3, then is REUSED for the conv3x3
    # results in phases 4-5 (y is dead once the pad interiors are built).
    pool_y = tc.alloc_tile_pool(name="ysb", bufs=1, side="right")
    y_sb = pool_y.tile([128, NHALF], BF16, name="y_sb")
    stats1 = tm.tile([128, NCH * 6], F32, name="stats1")
    yps = tc.alloc_tile_pool(name="yps", bufs=4, space="PSUM")
    for g in range(NCH):
        yp = yps.tile([128, CH], F32, name="yp", tag="yp")
        for s in range(S):
            pe.matmul(yp[0:C, :], lhsT=pT_hi[0:C, s * C:(s + 1) * C],
                      rhs=xs_at(s, 0, g), start=(s == 0), stop=(s == S - 1),
                      tile_position=(0, 0))
            pe.matmul(yp[64:64 + C, :], lhsT=pT_hi[64:64 + C, s * C:(s + 1) * C],
                      rhs=xs_at(s, 1, g), start=(s == 0), stop=(s == S - 1),
                      tile_position=(64, 64))
        dve.bn_stats(stats1[0:112, g * 6:(g + 1) * 6], yp[0:112, :])
        act.activation(y_sb[0:112, g * CH:(g + 1) * CH], yp[0:112, :], ACTF.Copy)
    yps.release()
    pool_xb.release()

    # padded conv inputs. rows 0-47: half-0 image rows -1..96 (slot = r+1);
    # rows 64-111: half-1 image rows 94..192 (slot = r-94). The border memset
    # overlaps the AllReduce; DVE fills interiors in phase 3.
    pool_pad = tc.alloc_tile_pool(name="pads", bufs=1, side="right")
    pad = pool_pad.tile([128, cfg.NPADG], BF16, name="pad")
    pad3 = pad[:].rearrange("p (h w) -> p h w", w=Wp)
    dve.memset(pad[0:112, :], 0.0)

    # ============ AllReduce 1 -> BN1 affine (gamma_cam folded in) ============
    s1, b1 = _bn_allreduce(nc, tc, tm, cfg, stats1, cc_in[0][:], cc_out[0][:],
                           groups, c_bn[:, 0:1], c_bn[:, 1:2], gamma=c_gvec)

    # ============ PHASE 3: pad-interior = relu(s1*y + b1) + X_a ============
    # chunk 47 first (half-0 rows 94/95 feed the row-group-64 halo), then 0..46.
    p3pool = tc.alloc_tile_pool(name="p3t", bufs=3, side="right")
    halo_done = False
    for g in [NCH - 1] + list(range(NCH - 1)):
        t = p3pool.tile([128, CH], F32, name="t3")
        act.activation(t[:], y_sb[:, g * CH:(g + 1) * CH], ACTF.Relu,
                       bias=b1[:, 0:1], scale=s1[:, 0:1])
        # half 0: image rows 2g,2g+1 -> slots 2g+1,2g+2
        dve.tensor_tensor(pad3[0:C, 2 * g + 1:2 * g + 3, 1:1 + W],
                          t[0:C, :].rearrange("p (a b) -> p a b", b=W),
                          xs_at(0, 0, g).rearrange("p (a b) -> p a b", b=W),
                          ALU.add)
        # half 1: image rows 96+2g,97+2g -> slots 2g+2,2g+3
        dve.tensor_tensor(pad3[64:64 + C, 2 * g + 2:2 * g + 4, 1:1 + W],
                          t[64:64 + C, :].rearrange("p (a b) -> p a b", b=W),
                          xs_at(0, 1, g).rearrange("p (a b) -> p a b", b=W),
                          ALU.add)
        if not halo_done and g == 0:
            # row-group 0 halo: image row 96 (half-1 slot 2) -> slot 97
            sync.dma_start(pad3[0:C, 97:98, :], pad3[64:64 + C, 2:3, :])
            # row-group 64 halo: image rows 94,95 (half-0 slots 95,96) -> 0,1
            sync.dma_start(pad3[64:64 + C, 0:2, :], pad3[0:C, 95:97, :])
            halo_done = True
    p3pool.release()
    pool_xa.release()

    # ============ PHASE 4: conv3x3 (9 shifted matmuls) + bn stats ============
    conv_sb = y_sb
    stats2 = tm.tile([128, NCH * 6], F32, name="stats2")
    cps = tc.alloc_tile_pool(name="cps", bufs=4, space="PSUM")
    for g in range(NCH):
        cp = cps.tile([128, CH], F32, name="cp", tag="cp")
        for t9 in range(9):
            dy, dx = t9 // 3, t9 % 3
            pe.matmul(cp[0:C, :], lhsT=c_taps[0:C, t9 * C:(t9 + 1) * C],
                      rhs=pad3[0:C, 2 * g + dy:2 * g + dy + 2, dx:dx + W],
                      start=(t9 == 0), stop=(t9 == 8), tile_position=(0, 0))
            pe.matmul(cp[64:64 + C, :], lhsT=c_taps[64:64 + C, t9 * C:(t9 + 1) * C],
                      rhs=pad3[64:64 + C, 2 * g + dy + 1:2 * g + dy + 3, dx:dx + W],
                      start=(t9 == 0), stop=(t9 == 8), tile_position=(64, 64))
        dve.bn_stats(stats2[0:112, g * 6:(g + 1) * 6], cp[0:112, :])
        act.activation(conv_sb[0:112, g * CH:(g + 1) * CH], cp[0:112, :], ACTF.Copy)
    cps.release()

    # ============ AllReduce 2 -> BN2 affine ============
    s2, b2 = _bn_allreduce(nc, tc, tm, cfg, stats2, cc_in[1][:], cc_out[1][:],
                           groups, c_bn[:, 2:3], c_bn[:, 3:4], gamma=None)

    # ============ PHASE 5: out = relu(s2*conv + b2) -> HBM (large stores) ======
    SB = 8                       # chunks per store block
    obp = tc.alloc_tile_pool(name="osb", bufs=2, side="right")
    for blk in range(NCH // SB):
        ob = obp.tile([128, SB * CH], F32, name="ob")
        for j in range(SB):
            g = blk * SB + j
            act.activation(ob[0:112, j * CH:(j + 1) * CH],
                           conv_sb[0:112, g * CH:(g + 1) * CH], ACTF.Relu,
                           bias=b2[0:112, 0:1], scale=s2[0:112, 0:1])
        b0 = blk * SB * CH
        ln = SB * CH
        hwq[blk % 2].dma_start(out_hbm[:, b0:b0 + ln], ob[0:C, :])
        hwq[(blk + 1) % 2].dma_start(out_hbm[:, NHALF + b0:NHALF + b0 + ln],
                                     ob[64:64 + C, :])
    obp.release()
    pool_pad.release()
    pool_y.release()
    tm.release()
    dpool.release()
    cpool.release()


def _bn_allreduce(nc, tc, tm, cfg, stats, cc_in, cc_out, groups, g_ap, b_ap,
                  gamma=None):
    """bn_stats blocks -> per-row (sum, sumsq) -> AllReduce over cores ->
    per-channel affine (scale, bias) replicated to rows 0-47 / 64-111."""
    C = cfg.C
    dve, act, gps, sync = nc.vector, nc.scalar, nc.gpsimd, nc.sync
    n_loc = cfg.NCH * cfg.ch
    inv_ntot = 1.0 / float(cfg.n_cores * 2 * n_loc)
    uid = "1" if gamma is not None else "2"

    aggr = tm.tile([128, 2], F32, name=f"aggr{uid}")
    dve.bn_aggr(aggr[0:C, :], stats[0:C, :])
    dve.bn_aggr(aggr[64:64 + C, :], stats[64:64 + C, :])
    ss = tm.tile([128, 2], F32, name=f"ss{uid}")
    dve.tensor_scalar_mul(ss[0:112, 0:1], aggr[0:112, 0:1], float(n_loc))
    msq = tm.tile([128, 1], F32, name=f"msq{uid}")
    dve.tensor_tensor(msq[0:112, :], aggr[0:112, 0:1], aggr[0:112, 0:1], ALU.mult)
    dve.tensor_tensor(ss[0:112, 1:2], aggr[0:112, 1:2], msq[0:112, :], ALU.add)
    dve.tensor_scalar_mul(ss[0:112, 1:2], ss[0:112, 1:2], float(n_loc))

    sync.dma_start(cc_in[:, 0:2], ss[0:C, :])
    sync.dma_start(cc_in[:, 2:4], ss[64:64 + C, :])
    gps.collective_compute("AllReduce", ALU.add, replica_groups=groups,
                           ins=[cc_in], outs=[cc_out])
    gsb = tm.tile([C, 4], F32, name=f"gsb{uid}")
    sync.dma_start(gsb[:], cc_out)

    mean = tm.tile([C, 1], F32, name=f"mean{uid}")
    dve.tensor_tensor(mean[:], gsb[:, 0:1], gsb[:, 2:3], ALU.add)
    dve.tensor_scalar_mul(mean[:], mean[:], inv_ntot)
    ex2 = tm.tile([C, 1], F32, name=f"ex2{uid}")
    dve.tensor_tensor(ex2[:], gsb[:, 1:2], gsb[:, 3:4], ALU.add)
    dve.tensor_scalar_mul(ex2[:], ex2[:], inv_ntot)
    msq2 = tm.tile([C, 1], F32, name=f"msq2{uid}")
    dve.tensor_tensor(msq2[:], mean[:], mean[:], ALU.mult)
    var = tm.tile([C, 1], F32, name=f"var{uid}")
    dve.tensor_tensor(var[:], ex2[:], msq2[:], ALU.subtract)
    dve.tensor_scalar_add(var[:], var[:], cfg.eps)
    sd = tm.tile([C, 1], F32, name=f"sd{uid}")
    act.activation(sd[:], var[:], ACTF.Sqrt)
    inv = tm.tile([C, 1], F32, name=f"inv{uid}")
    dve.reciprocal(inv[:], sd[:])

    sc = tm.tile([128, 1], F32, name=f"sc{uid}")
    bi = tm.tile([128, 1], F32, name=f"bi{uid}")
    gps.memset(sc[:], 0.0)
    gps.memset(bi[:], 0.0)
    dve.tensor_tensor(sc[0:C, :], inv[:], g_ap, ALU.mult)
    bt = tm.tile([C, 1], F32, name=f"bt{uid}")
    dve.tensor_tensor(bt[:], mean[:], sc[0:C, :], ALU.mult)
    dve.tensor_tensor(bi[0:C, :], b_ap, bt[:], ALU.subtract)
    if gamma is not None:        # fold gamma_cam (valid for gamma_cam >= 0)
        dve.tensor_tensor(sc[0:C, :], sc[0:C, :], gamma[0:C, :], ALU.mult)
        dve.tensor_tensor(bi[0:C, :], bi[0:C, :], gamma[0:C, :], ALU.mult)
    sync.dma_start(sc[64:64 + C, :], sc[0:C, :])
    sync.dma_start(bi[64:64 + C, :], bi[0:C, :])
    return sc, bi


# ======================= host side =======================

_CACHE = {}


def _prep_consts(cfg, inputs):
    C = cfg.C
    f32 = np.float32
    Wm = [np.asarray(inputs[k], f32) for k in ("Wa", "Wb", "Wc", "Wd")]
    fuse_w = np.asarray(inputs["fuse_w"], f32)[:, :, 0, 0]
    out_w = np.asarray(inputs["out_w"], f32)
    taps = np.zeros((128, 9 * C), np.float32)
    for t in range(9):
        dy, dx = t // 3, t % 3
        wT = out_w[:, :, dy, dx].T
        taps[0:C, t * C:(t + 1) * C] = wT
        taps[64:64 + C, t * C:(t + 1) * C] = wT
    bn_gb = np.stack([np.asarray(inputs["fuse_gamma"], f32),
                      np.asarray(inputs["fuse_beta"], f32),
                      np.asarray(inputs["out_gamma"], f32),
                      np.asarray(inputs["out_beta"], f32)], axis=1)
    gvec = np.full((128, 1), np.asarray(inputs["gamma_cam"], f32).reshape(-1)[0], f32)
    return {
        "waT": np.ascontiguousarray(Wm[0].T),
        "wcatT": np.ascontiguousarray(np.concatenate([w.T for w in Wm], 1)),
        "wcat": np.ascontiguousarray(np.concatenate(Wm, 1)),
        "fcatT": np.ascontiguousarray(
            np.concatenate([fuse_w[:, s * C:(s + 1) * C].T for s in range(4)], 1)),
        "eye": np.eye(C, dtype=f32),
        "tapsT": taps.astype(ml_dtypes.bfloat16),
        "bn_gb": np.ascontiguousarray(bn_gb),
        "gvec": gvec,
    }


def _make_in_maps(cfg, inputs):
    consts = _prep_consts(cfg, inputs)
    B, C, H, W = cfg.n_cores, cfg.C, cfg.H, cfg.W
    xs = [np.asarray(inputs[k], np.float32).reshape(B, C, H * W)
          for k in ("input_feature", "fb", "fc", "fd")]
    xcat = np.concatenate(xs, axis=1).astype(ml_dtypes.bfloat16)  # [B, 4C, N]
    in_maps = []
    for b in range(B):
        m = dict(consts)
        m["xcat"] = np.ascontiguousarray(xcat[b])
        in_maps.append(m)
    return in_maps


def _get_built(cfg):
    if cfg not in _CACHE:
        nc = bacc.Bacc("TRN2", target_bir_lowering=False, debug=False,
                       enable_asserts=False, num_devices=cfg.n_cores)
        _CACHE[cfg] = build_kernel(nc, cfg)
    return _CACHE[cfg]


def kernel(**inputs):
    from concourse import bass_utils
    cfg = Cfg()
    nc = _get_built(cfg)
    in_maps = _make_in_maps(cfg, inputs)
    B, C, H, W = cfg.n_cores, cfg.C, cfg.H, cfg.W
    res = bass_utils.run_bass_kernel_spmd(nc, in_maps, core_ids=list(range(B)))
    out = np.stack([res.results[b]["out"].reshape(C, H, W) for b in range(B)])
    return out.astype(np.float32)


if __name__ == "__main__":
    _get_built(Cfg())
    print("built OK")
